# revision 8
# baseline (speedup 1.0000x reference)
"""MPNCOV (iSQRT-COV pooling) Trainium2 kernel.

Math per sample (C=256 channels, M=196 spatial):
  xc   = x - mean_m(x)                      # center along spatial dim
  A    = xc @ xc^T / (M * tr),  tr = sum(xc^2)   (= cov / trace(cov))
  Newton-Schulz (ITER_N=3) on A, final y = sqrt(tr) * YZY, triu-packed.

Scale folding: every intermediate X is stored as X_s with X = sigma_X * X_s,
sigma tracked symbolically so each PSUM->SBUF transform is a single
tensor_tensor subtract against a constant diagonal tile:
  ZY1_s = 3I   - A_s          (sigma 1/2)
  Y1_s  = A_s @ ZY1_s         (sigma 1/2)
  W1_s  = ZY1_s @ Y1_s        (sigma 1/4)
  ZY2_s = 12I  - W1_s         (sigma 1/8)
  Y2_s  = Y1_s @ ZY2_s        (sigma 1/16)
  Z2_s  = ZY2_s @ ZY1_s       (sigma 1/16)
  W2_s  = Z2_s @ Y2_s         (sigma 1/256)
  ZY3_s = 768I - W2_s
  F_s   = Y2_s @ ZY3_s,   y = (sqrt(tr)/8192) * F_s
All intermediates are polynomials in symmetric A => symmetric, so row-tiles
serve directly as matmul lhsT (no transposes in the NS chain). The only PE
transposes build xc^T for the Gram matmul; 1/sqrt(M*tr) is folded into the
transpose's PSUM->SBUF copy so the Gram directly yields A_s.

Sharding: pure data parallel, batch 256 -> 32 samples on each of 8 cores.
Triu packing: all 32 per-sample results stay SBUF-resident (2 sample-groups),
then one DMA per matrix row r moves that row's triu tail for all 16 samples
of a group (constant strides in both src and dst).
"""

import numpy as np

from concourse import bacc, bass, mybir, tile
from concourse import bass_utils

F32 = mybir.dt.float32
P = 128
C = 256
M = 196
B = 256
NCORES = 8
S = B // NCORES            # samples per core
NTRIU = C * (C + 1) // 2   # 32896
GROUPS = 2
GS = S // GROUPS           # samples per output group

# matmul input dtype for the big products (float32r = relaxed-precision fp32
# path, 4x faster at N>=256). Set to F32 for the exact baseline.
MM_DT = mybir.dt.float32r

LAST_EXEC_NS = None        # set when kernel() is called with _trace=True
LAST_RESULTS = None


def _mm(nc, out, lhsT, rhs, start, stop):
    nc.tensor.matmul(out, lhsT, rhs, start=start, stop=stop)


def build(tc, y_ap, x_ap, ident_ap, icons_ap, ones_ap, onesrow_ap, n_samples=S):
    nc = tc.nc
    import contextlib

    with contextlib.ExitStack() as ctx:
        consts = ctx.enter_context(tc.tile_pool(name="consts", bufs=1))
        fpool = ctx.enter_context(tc.tile_pool(name="fpool", bufs=1))
        work = ctx.enter_context(tc.tile_pool(name="work", bufs=3))
        mats = ctx.enter_context(tc.tile_pool(name="mats", bufs=2))
        psum = ctx.enter_context(tc.tile_pool(name="psum", bufs=6, space="PSUM"))
        psum_small = ctx.enter_context(
            tc.tile_pool(name="psum_small", bufs=2, space="PSUM")
        )

        # constants
        ident = consts.tile([P, P], F32, tag="ident")
        nc.sync.dma_start(ident[:], ident_ap[:])
        icons = consts.tile([P, 6, C], F32, tag="icons")
        nc.sync.dma_start(icons[:], icons_ap[:])
        ones = consts.tile([P, 1], F32, tag="ones")
        nc.sync.dma_start(ones[:], ones_ap[:])
        onesrow = consts.tile([1, P], F32, tag="onesrow")
        nc.sync.dma_start(onesrow[:], onesrow_ap[:])

        n_groups = (n_samples + GS - 1) // GS
        # per-group per-mtile result stores: [P, GS, C]
        ftiles = [
            [
                fpool.tile(
                    [P, GS, C], F32, tag=f"F_g{g}_m{mt}", name=f"F_g{g}_m{mt}"
                )
                for mt in range(2)
            ]
            for g in range(n_groups)
        ]

        rowstart = np.concatenate([[0], np.cumsum(C - np.arange(C))]).astype(np.int64)

        for b in range(n_samples):
            g, bi = b // GS, b % GS

            # ---- load sample: [256, 196] -> [128, 2, 196] (c = h*128 + p) ----
            xr = work.tile([P, 2, M], F32, tag="xr")
            nc.sync.dma_start(xr[:], x_ap[b].rearrange("(h p) m -> p h m", p=P))

            # ---- center + squared-row-sums ----
            mean2 = work.tile([P, 2], F32, tag="mean2")
            nc.vector.tensor_reduce(
                mean2[:], xr[:], axis=mybir.AxisListType.X, op=mybir.AluOpType.add
            )
            negmean = work.tile([P, 2], F32, tag="negmean")
            nc.vector.tensor_scalar_mul(negmean[:], mean2[:], -1.0 / M)
            xc = work.tile([P, 2, M], F32, tag="xc")
            sq = work.tile([P, 2, M], F32, tag="sq")
            s2 = work.tile([P, 2], F32, tag="s2")
            for h in range(2):
                nc.scalar.activation(
                    xc[:, h],
                    xr[:, h],
                    mybir.ActivationFunctionType.Identity,
                    bias=negmean[:, h : h + 1],
                )
                nc.scalar.activation(
                    sq[:, h],
                    xc[:, h],
                    mybir.ActivationFunctionType.Square,
                    accum_out=s2[:, h : h + 1],
                )

            # ---- trace -> alpha = 1/sqrt(M*tr), beta = sqrt(tr)/8192 ----
            tr_ps = psum_small.tile([1, 1], F32, tag="ps_small")
            nc.tensor.matmul(tr_ps[:], s2[:, 0:1], ones[:], start=True, stop=False)
            nc.tensor.matmul(tr_ps[:], s2[:, 1:2], ones[:], start=False, stop=True)
            ab = work.tile([1, 2], F32, tag="ab")
            inv = work.tile([1, 1], F32, tag="inv")
            nc.vector.reciprocal(inv[:], tr_ps[:])
            # s2 sums give tr_ps = sum(xc^2) = M * trace(cov) = M * normA.
            # alpha^2 must equal 1/(M*normA) = 1/tr_ps -> alpha = sqrt(1/tr_ps)
            # beta = sqrt(normA)/8192 = sqrt(tr_ps / M) / 8192
            nc.scalar.activation(
                ab[:, 0:1], inv[:], mybir.ActivationFunctionType.Sqrt, scale=1.0
            )
            nc.scalar.activation(
                ab[:, 1:2],
                tr_ps[:],
                mybir.ActivationFunctionType.Sqrt,
                scale=1.0 / (M * 8192.0 * 8192.0),
            )
            bc_ps = psum_small.tile([P, 2], F32, tag="ps_small")
            nc.tensor.matmul(bc_ps[:], onesrow[:], ab[:], start=True, stop=True)
            abv = work.tile([P, 2], F32, tag="abv")
            nc.any.tensor_copy(abv[:], bc_ps[:])

            # ---- transpose xc -> xcT (two k-chunks), scaled by alpha ----
            tp0 = psum.tile([P, C], F32, tag="ps_big")
            tp1 = psum.tile([P, C], F32, tag="ps_big")
            for h in range(2):
                cs = slice(h * P, (h + 1) * P)
                nc.tensor.transpose(tp0[:, cs], xc[:, h, 0:P], ident[:])
                nc.tensor.transpose(tp1[0 : M - P, cs], xc[:, h, P:M], ident[:])
            xcT0 = work.tile([P, C], MM_DT, tag="xcT0")
            xcT1 = work.tile([P, C], MM_DT, tag="xcT1")
            nc.scalar.activation(
                xcT0[:],
                tp0[:],
                mybir.ActivationFunctionType.Copy,
                scale=abv[:, 0:1],
            )
            nc.scalar.activation(
                xcT1[0 : M - P],
                tp1[0 : M - P],
                mybir.ActivationFunctionType.Copy,
                scale=abv[0 : M - P, 0:1],
            )

            # ---- Gram -> A_s ----
            def prod(U0, U1, V0, V1, n=C):
                """psum pair <- [U0;U1]^T-rows @ V   (U symmetric => U@V)."""
                ps = []
                for mt in range(2):
                    ms = slice(mt * P, (mt + 1) * P)
                    p_t = psum.tile([P, n], F32, tag="ps_big")
                    _mm(nc, p_t[:], U0[:, ms], V0[:], True, False)
                    _mm(nc, p_t[:], U1[:, ms], V1[:], False, True)
                    ps.append(p_t)
                return ps

            def gram():
                ps = []
                for mt in range(2):
                    ms = slice(mt * P, (mt + 1) * P)
                    p_t = psum.tile([P, C], F32, tag="ps_big")
                    _mm(nc, p_t[:], xcT0[:, ms], xcT0[:], True, False)
                    _mm(
                        nc,
                        p_t[:],
                        xcT1[0 : M - P, ms],
                        xcT1[0 : M - P, :],
                        False,
                        True,
                    )
                    ps.append(p_t)
                return ps

            def sub_const(ps_pair, k, tag):
                """sbuf pair <- icons[k+mt] - psum  (ZY transform)."""
                out = []
                for mt in range(2):
                    t = mats.tile([P, C], MM_DT, tag=f"{tag}{mt}", name=f"{tag}{mt}")
                    nc.vector.tensor_tensor(
                        t[:], icons[:, k + mt, :], ps_pair[mt][:],
                        op=mybir.AluOpType.subtract,
                    )
                    out.append(t)
                return out

            def to_sbuf(ps_pair, tag):
                out = []
                for mt in range(2):
                    t = mats.tile([P, C], MM_DT, tag=f"{tag}{mt}", name=f"{tag}{mt}")
                    nc.any.tensor_copy(t[:], ps_pair[mt][:])
                    out.append(t)
                return out

            a_ps = gram()
            A = to_sbuf(a_ps, "A")
            ZY1 = sub_const(a_ps, 0, "ZY1")

            y1_ps = prod(A[0], A[1], ZY1[0], ZY1[1])
            Y1 = to_sbuf(y1_ps, "Y1")

            w1_ps = prod(ZY1[0], ZY1[1], Y1[0], Y1[1])
            ZY2 = sub_const(w1_ps, 2, "ZY2")

            y2_ps = prod(Y1[0], Y1[1], ZY2[0], ZY2[1])
            Y2 = to_sbuf(y2_ps, "Y2")

            z2_ps = prod(ZY2[0], ZY2[1], ZY1[0], ZY1[1])
            Z2 = to_sbuf(z2_ps, "Z2")

            w2_ps = prod(Z2[0], Z2[1], Y2[0], Y2[1])
            ZY3 = sub_const(w2_ps, 4, "ZY3")

            f_ps = prod(Y2[0], Y2[1], ZY3[0], ZY3[1])
            for mt in range(2):
                nc.scalar.activation(
                    ftiles[g][mt][:, bi, :],
                    f_ps[mt][:],
                    mybir.ActivationFunctionType.Copy,
                    scale=abv[:, 1:2],
                )

            # ---- flush group: one DMA per matrix row, all samples in group ----
            if bi == GS - 1 or b == n_samples - 1:
                b0 = g * GS
                ng = bi + 1
                for r in range(C):
                    L = C - r
                    s0 = int(rowstart[r])
                    src = ftiles[g][r // P][r % P : r % P + 1, 0:ng, r:C]
                    eng = nc.sync if r % 2 == 0 else nc.scalar
                    eng.dma_start(y_ap[b0 : b0 + ng, s0 : s0 + L], src)


def _make_const_inputs():
    e0 = np.zeros((P, C), np.float32)
    e0[np.arange(P), np.arange(P)] = 1.0
    e1 = np.zeros((P, C), np.float32)
    e1[np.arange(P), P + np.arange(P)] = 1.0
    icons = np.stack(
        [3.0 * e0, 3.0 * e1, 12.0 * e0, 12.0 * e1, 768.0 * e0, 768.0 * e1], axis=1
    )  # [P, 6, C]
    return {
        "ident": np.eye(P, dtype=np.float32),
        "icons": np.ascontiguousarray(icons),
        "ones": np.ones((P, 1), np.float32),
        "onesrow": np.ones((1, P), np.float32),
    }


def make_nc(n_samples=S, num_devices=NCORES):
    nc = bacc.Bacc(
        "TRN2",
        target_bir_lowering=False,
        debug=False,
        enable_asserts=False,
        num_devices=num_devices,
    )
    x_ap = nc.dram_tensor("x", (n_samples, C, M), F32, kind="ExternalInput").ap()
    y_ap = nc.dram_tensor("y", (n_samples, NTRIU), F32, kind="ExternalOutput").ap()
    ident_ap = nc.dram_tensor("ident", (P, P), F32, kind="ExternalInput").ap()
    icons_ap = nc.dram_tensor("icons", (P, 6, C), F32, kind="ExternalInput").ap()
    ones_ap = nc.dram_tensor("ones", (P, 1), F32, kind="ExternalInput").ap()
    onesrow_ap = nc.dram_tensor("onesrow", (1, P), F32, kind="ExternalInput").ap()
    with tile.TileContext(nc) as tc:
        build(tc, y_ap, x_ap, ident_ap, icons_ap, ones_ap, onesrow_ap, n_samples)
    nc.compile()
    return nc


def kernel(x, _trace=False, **_trace_kwargs):
    global LAST_EXEC_NS, LAST_RESULTS
    x = np.ascontiguousarray(np.asarray(x), dtype=np.float32)
    assert x.shape == (B, C, 14, 14)
    xr = x.reshape(B, C, M)

    nc = make_nc()
    consts = _make_const_inputs()
    in_maps = [
        {"x": np.ascontiguousarray(xr[i * S : (i + 1) * S]), **consts}
        for i in range(NCORES)
    ]
    res = bass_utils.run_bass_kernel_spmd(
        nc, in_maps, core_ids=list(range(NCORES)), trace=_trace, **_trace_kwargs
    )
    LAST_EXEC_NS = res.exec_time_ns
    LAST_RESULTS = res
    return np.concatenate([r["y"] for r in res.results], axis=0)


# revision 10
# speedup vs baseline: 1.0263x; 1.0263x over previous
"""MPNCOV (iSQRT-COV pooling) Trainium2 kernel.

Math per sample (C=256 channels, M=196 spatial):
  xc   = x - mean_m(x)                      # center along spatial dim
  A    = xc @ xc^T / sum(xc^2)              # = cov / trace(cov)
  Newton-Schulz (ITER_N=3) on A, final y = sqrt(normA) * YZY, triu-packed.

Scale folding: every intermediate X is stored as X_s with X = sigma_X * X_s,
sigma tracked symbolically so each PSUM->SBUF transform is a single
tensor_tensor subtract against a constant diagonal tile:
  ZY1_s = 3I   - A_s          (sigma 1/2)
  Y1_s  = A_s @ ZY1_s         (sigma 1/2)
  W1_s  = ZY1_s @ Y1_s        (sigma 1/4)
  ZY2_s = 12I  - W1_s         (sigma 1/8)
  Y2_s  = Y1_s @ ZY2_s        (sigma 1/16)
  Z2_s  = ZY2_s @ ZY1_s       (sigma 1/16)
  W2_s  = Z2_s @ Y2_s         (sigma 1/256)
  ZY3_s = 768I - W2_s
  F_s   = Y2_s @ ZY3_s,   y = (sqrt(tr/M)/8192) * F_s
All intermediates are polynomials in symmetric A => symmetric, so row-tiles
serve directly as matmul lhsT (no transposes in the NS chain). The only PE
transposes build xc^T for the Gram matmul; 1/sqrt(sum xc^2) is folded into
the transpose's PSUM->SBUF copy so the Gram directly yields A_s.

Matrices are stored as single [128, 512] tiles: cols 0:256 = matrix rows
0:128, cols 256:512 = matrix rows 128:256. Each product lands in ONE fp32
PSUM bank [128, 512] (two N=256 matmul groups), so every PSUM->SBUF
transform is one 512-wide DVE/ACT op. Matmul inputs are fp16 (1 cyc/row on
the PE + fast weight load); PSUM accumulation stays fp32.

Sharding: pure data parallel, batch 256 -> 32 samples on each of 8 cores.
Triu packing: all 32 per-sample results stay SBUF-resident; at the end one
DMA per matrix row r moves that row's triu tail for all 32 samples
(constant strides in both src and dst), alternating sync/scalar HWDGE.
"""

import numpy as np

from concourse import bacc, bass, mybir, tile
from concourse import bass_utils

F32 = mybir.dt.float32
P = 128
C = 256
M = 196
B = 256
NCORES = 8
S = B // NCORES            # samples per core
NTRIU = C * (C + 1) // 2   # 32896

# matmul input dtype for the big products
MM_DT = mybir.dt.float16

LAST_EXEC_NS = None
LAST_RESULTS = None


def build(tc, y_ap, x_ap, ident_ap, icons_ap, ones_ap, onesrow_ap, n_samples=S):
    nc = tc.nc
    import contextlib

    with contextlib.ExitStack() as ctx:
        consts = ctx.enter_context(tc.tile_pool(name="consts", bufs=1))
        fpool = ctx.enter_context(tc.tile_pool(name="fpool", bufs=1))
        work = ctx.enter_context(tc.tile_pool(name="work", bufs=3))
        mats = ctx.enter_context(tc.tile_pool(name="mats", bufs=2))
        psum = ctx.enter_context(tc.tile_pool(name="psum", bufs=6, space="PSUM"))
        psum_small = ctx.enter_context(
            tc.tile_pool(name="psum_small", bufs=2, space="PSUM")
        )

        ident = consts.tile([P, P], MM_DT, tag="ident")
        nc.sync.dma_start(ident[:], ident_ap[:])
        icons = consts.tile([P, 3, 2 * C], F32, tag="icons")
        nc.sync.dma_start(icons[:], icons_ap[:])
        ones = consts.tile([P, 1], F32, tag="ones")
        nc.sync.dma_start(ones[:], ones_ap[:])
        onesrow = consts.tile([1, P], F32, tag="onesrow")
        nc.sync.dma_start(onesrow[:], onesrow_ap[:])

        ftiles = [
            fpool.tile([P, n_samples, C], F32, tag=f"F_m{mt}", name=f"F_m{mt}")
            for mt in range(2)
        ]

        rowstart = np.concatenate([[0], np.cumsum(C - np.arange(C))]).astype(np.int64)

        def prod(U, V):
            """One [128,512] PSUM bank <- U @ V (both [P,512] fp16, symmetric)."""
            p_t = psum.tile([P, 2 * C], F32, tag="ps_big")
            for mt in range(2):
                oc = slice(mt * C, (mt + 1) * C)
                ms0 = slice(mt * P, mt * P + P)
                ms1 = slice(C + mt * P, C + mt * P + P)
                nc.tensor.matmul(
                    p_t[:, oc], U[:, ms0], V[:, 0:C], start=True, stop=False
                )
                nc.tensor.matmul(
                    p_t[:, oc], U[:, ms1], V[:, C : 2 * C], start=False, stop=True
                )
            return p_t

        for b in range(n_samples):
            # ---- load sample: [256, 196] -> [128, 2, 196] (c = h*128 + p) ----
            xr = work.tile([P, 2, M], F32, tag="xr")
            nc.sync.dma_start(xr[:], x_ap[b].rearrange("(h p) m -> p h m", p=P))

            # ---- center (fp16 out) + squared-row-sums ----
            mean2 = work.tile([P, 2], F32, tag="mean2")
            nc.vector.tensor_reduce(
                mean2[:], xr[:], axis=mybir.AxisListType.X, op=mybir.AluOpType.add
            )
            negmean = work.tile([P, 2], F32, tag="negmean")
            nc.vector.tensor_scalar_mul(negmean[:], mean2[:], -1.0 / M)
            xc = work.tile([P, 2, M], MM_DT, tag="xc")
            sq = work.tile([P, 2, M], MM_DT, tag="sq")
            s2 = work.tile([P, 2], F32, tag="s2")
            for h in range(2):
                nc.scalar.activation(
                    xc[:, h],
                    xr[:, h],
                    mybir.ActivationFunctionType.Identity,
                    bias=negmean[:, h : h + 1],
                )
                nc.scalar.activation(
                    sq[:, h],
                    xc[:, h],
                    mybir.ActivationFunctionType.Square,
                    accum_out=s2[:, h : h + 1],
                )

            # ---- trace: tr_ps = sum(xc^2) = M * normA ----
            # alpha = sqrt(1/tr_ps); beta = sqrt(tr_ps/M)/8192
            tr_ps = psum_small.tile([1, 1], F32, tag="ps_small")
            nc.tensor.matmul(tr_ps[:], s2[:, 0:1], ones[:], start=True, stop=False)
            nc.tensor.matmul(tr_ps[:], s2[:, 1:2], ones[:], start=False, stop=True)
            ab = work.tile([1, 2], F32, tag="ab")
            inv = work.tile([1, 1], F32, tag="inv")
            nc.vector.reciprocal(inv[:], tr_ps[:])
            nc.scalar.activation(
                ab[:, 0:1], inv[:], mybir.ActivationFunctionType.Sqrt, scale=1.0
            )
            nc.scalar.activation(
                ab[:, 1:2],
                tr_ps[:],
                mybir.ActivationFunctionType.Sqrt,
                scale=1.0 / (M * 8192.0 * 8192.0),
            )
            bc_ps = psum_small.tile([P, 2], F32, tag="ps_small")
            nc.tensor.matmul(bc_ps[:], onesrow[:], ab[:], start=True, stop=True)
            abv = work.tile([P, 2], F32, tag="abv")
            nc.vector.tensor_copy(abv[:], bc_ps[:])

            # ---- transpose xc into one bank: cols 0:256 = k0, 256:512 = k1 ----
            tp = psum.tile([P, 2 * C], MM_DT, tag="ps_big")
            for h in range(2):
                nc.tensor.transpose(tp[:, h * P : h * P + P], xc[:, h, 0:P], ident[:])
                nc.tensor.transpose(
                    tp[0 : M - P, C + h * P : C + h * P + P], xc[:, h, P:M], ident[:]
                )
            xcT0 = work.tile([P, C], MM_DT, tag="xcT0")
            xcT1 = work.tile([P, C], MM_DT, tag="xcT1")
            nc.vector.tensor_scalar_mul(xcT0[:], tp[:, 0:C], abv[:, 0:1])
            nc.vector.tensor_scalar_mul(
                xcT1[0 : M - P], tp[0 : M - P, C : 2 * C], abv[0 : M - P, 0:1]
            )

            # ---- Gram -> A_s (one bank) ----
            a_ps = psum.tile([P, 2 * C], F32, tag="ps_big")
            for mt in range(2):
                oc = slice(mt * C, (mt + 1) * C)
                ms = slice(mt * P, (mt + 1) * P)
                nc.tensor.matmul(
                    a_ps[:, oc], xcT0[:, ms], xcT0[:], start=True, stop=False
                )
                nc.tensor.matmul(
                    a_ps[:, oc],
                    xcT1[0 : M - P, ms],
                    xcT1[0 : M - P, :],
                    start=False,
                    stop=True,
                )

            def sub_const(ps, k, tag, eng):
                t = mats.tile([P, 2 * C], MM_DT, tag=tag, name=tag)
                eng.tensor_tensor(
                    t[:], icons[:, k, :], ps[:], op=mybir.AluOpType.subtract
                )
                return t

            A = mats.tile([P, 2 * C], MM_DT, tag="A", name="A")
            nc.vector.tensor_copy(A[:], a_ps[:])
            ZY1 = sub_const(a_ps, 0, "ZY1", nc.vector)

            y1_ps = prod(A, ZY1)
            Y1 = mats.tile([P, 2 * C], MM_DT, tag="Y1", name="Y1")
            nc.vector.tensor_copy(Y1[:], y1_ps[:])

            w1_ps = prod(ZY1, Y1)
            ZY2 = sub_const(w1_ps, 1, "ZY2", nc.vector)

            y2_ps = prod(Y1, ZY2)
            Y2 = mats.tile([P, 2 * C], MM_DT, tag="Y2", name="Y2")
            nc.vector.tensor_copy(Y2[:], y2_ps[:])

            z2_ps = prod(ZY2, ZY1)
            Z2 = mats.tile([P, 2 * C], MM_DT, tag="Z2", name="Z2")
            nc.scalar.activation(
                Z2[:], z2_ps[:], mybir.ActivationFunctionType.Copy
            )

            w2_ps = prod(Z2, Y2)
            ZY3 = sub_const(w2_ps, 2, "ZY3", nc.vector)

            f_ps = prod(Y2, ZY3)
            for mt in range(2):
                nc.vector.tensor_scalar_mul(
                    ftiles[mt][:, b, :], f_ps[:, mt * C : (mt + 1) * C], abv[:, 1:2]
                )

        # ---- flush: one DMA per matrix row, all samples at once ----
        for r in range(C):
            L = C - r
            s0 = int(rowstart[r])
            src = ftiles[r // P][r % P : r % P + 1, :, r:C]
            eng = nc.sync if r % 2 == 0 else nc.scalar
            eng.dma_start(y_ap[:, s0 : s0 + L], src)


def _make_const_inputs():
    # icons[:, k, :]: [3I, 12I, 768I] in concatenated row-tile layout:
    # cols 0:256 = matrix rows 0:128 (diag at col p),
    # cols 256:512 = matrix rows 128:256 (diag at col 256+128+p).
    e = np.zeros((P, 2 * C), np.float32)
    e[np.arange(P), np.arange(P)] = 1.0
    e[np.arange(P), C + P + np.arange(P)] = 1.0
    icons = np.stack([3.0 * e, 12.0 * e, 768.0 * e], axis=1)  # [P, 3, 512]
    return {
        "ident": np.eye(P, dtype=np.float16),
        "icons": np.ascontiguousarray(icons),
        "ones": np.ones((P, 1), np.float32),
        "onesrow": np.ones((1, P), np.float32),
    }


def make_nc(n_samples=S, num_devices=NCORES):
    nc = bacc.Bacc(
        "TRN2",
        target_bir_lowering=False,
        debug=False,
        enable_asserts=False,
        num_devices=num_devices,
    )
    x_ap = nc.dram_tensor("x", (n_samples, C, M), F32, kind="ExternalInput").ap()
    y_ap = nc.dram_tensor("y", (n_samples, NTRIU), F32, kind="ExternalOutput").ap()
    ident_ap = nc.dram_tensor("ident", (P, P), MM_DT, kind="ExternalInput").ap()
    icons_ap = nc.dram_tensor("icons", (P, 3, 2 * C), F32, kind="ExternalInput").ap()
    ones_ap = nc.dram_tensor("ones", (P, 1), F32, kind="ExternalInput").ap()
    onesrow_ap = nc.dram_tensor("onesrow", (1, P), F32, kind="ExternalInput").ap()
    with tile.TileContext(nc) as tc:
        build(tc, y_ap, x_ap, ident_ap, icons_ap, ones_ap, onesrow_ap, n_samples)
    nc.compile()
    return nc


def kernel(x, _trace=False, **_trace_kwargs):
    global LAST_EXEC_NS, LAST_RESULTS
    x = np.ascontiguousarray(np.asarray(x), dtype=np.float32)
    assert x.shape == (B, C, 14, 14)
    xr = x.reshape(B, C, M)

    nc = make_nc()
    consts = _make_const_inputs()
    in_maps = [
        {"x": np.ascontiguousarray(xr[i * S : (i + 1) * S]), **consts}
        for i in range(NCORES)
    ]
    res = bass_utils.run_bass_kernel_spmd(
        nc, in_maps, core_ids=list(range(NCORES)), trace=_trace, **_trace_kwargs
    )
    LAST_EXEC_NS = res.exec_time_ns
    LAST_RESULTS = res
    return np.concatenate([r["y"] for r in res.results], axis=0)


# revision 13
# speedup vs baseline: 1.6460x; 1.6038x over previous
"""MPNCOV (iSQRT-COV pooling) Trainium2 kernel.

Math per sample (C=256 channels, M=196 spatial):
  xc   = x - mean_m(x)                      # center along spatial dim
  A    = xc @ xc^T / sum(xc^2)              # = cov / trace(cov)
  Newton-Schulz (ITER_N=3) on A, final y = sqrt(normA) * YZY, triu-packed.

Scale folding: every intermediate X is stored as X_s with X = sigma_X * X_s,
sigma tracked symbolically so each PSUM->SBUF transform is a single
tensor_tensor subtract against a constant diagonal tile:
  ZY1_s = 3I   - A_s          (sigma 1/2)
  Y1_s  = A_s @ ZY1_s         (sigma 1/2)
  W1_s  = ZY1_s @ Y1_s        (sigma 1/4)
  ZY2_s = 12I  - W1_s         (sigma 1/8)
  Y2_s  = Y1_s @ ZY2_s        (sigma 1/16)
  Z2_s  = ZY2_s @ ZY1_s       (sigma 1/16)
  W2_s  = Z2_s @ Y2_s         (sigma 1/256)
  ZY3_s = 768I - W2_s
  F_s   = Y2_s @ ZY3_s,   y = (sqrt(tr/M)/8192) * F_s
All intermediates are polynomials in symmetric A => symmetric, so row-tiles
serve directly as matmul lhsT (no transposes in the NS chain). The only PE
transposes build xc^T for the Gram matmul; 1/sqrt(sum xc^2) is folded into
the transpose's PSUM->SBUF copy so the Gram directly yields A_s.

Matrices are stored as single [128, 512] tiles: cols 0:256 = matrix rows
0:128, cols 256:512 = matrix rows 128:256. Each product lands in ONE fp32
PSUM bank [128, 512] (two N=256 matmul groups), so every PSUM->SBUF
transform is one 512-wide DVE/ACT op. Matmul inputs are fp16 (1 cyc/row on
the PE + fast weight load); PSUM accumulation stays fp32.

Sharding: pure data parallel, batch 256 -> 32 samples on each of 8 cores.
Triu packing: all 32 per-sample results stay SBUF-resident; at the end one
DMA per matrix row r moves that row's triu tail for all 32 samples
(constant strides in both src and dst), alternating sync/scalar HWDGE.
"""

import numpy as np

from concourse import bacc, bass, mybir, tile
from concourse import bass_utils

F32 = mybir.dt.float32
P = 128
C = 256
M = 196
B = 256
NCORES = 8
S = B // NCORES            # samples per core
NTRIU = C * (C + 1) // 2   # 32896

# matmul input dtype for the big products
MM_DT = mybir.dt.float16

LAST_EXEC_NS = None
LAST_RESULTS = None


def build(tc, y_ap, x_ap, ident_ap, icons_ap, ones_ap, onesrow_ap, n_samples=S):
    nc = tc.nc
    import contextlib

    with contextlib.ExitStack() as ctx:
        consts = ctx.enter_context(tc.tile_pool(name="consts", bufs=1))
        fpool = ctx.enter_context(tc.tile_pool(name="fpool", bufs=1))
        work = ctx.enter_context(tc.tile_pool(name="work", bufs=3))
        mats = ctx.enter_context(tc.tile_pool(name="mats", bufs=2))
        psum = ctx.enter_context(tc.tile_pool(name="psum", bufs=6, space="PSUM"))
        psum_small = ctx.enter_context(
            tc.tile_pool(name="psum_small", bufs=2, space="PSUM")
        )

        ident = consts.tile([P, P], MM_DT, tag="ident")
        nc.sync.dma_start(ident[:], ident_ap[:])
        icons = consts.tile([P, 3, 2 * C], MM_DT, tag="icons")
        nc.sync.dma_start(icons[:], icons_ap[:])
        ones = consts.tile([P, 1], F32, tag="ones")
        nc.sync.dma_start(ones[:], ones_ap[:])
        onesrow = consts.tile([1, P], F32, tag="onesrow")
        nc.sync.dma_start(onesrow[:], onesrow_ap[:])

        ftiles = [
            fpool.tile([P, n_samples, C], F32, tag=f"F_m{mt}", name=f"F_m{mt}")
            for mt in range(2)
        ]

        rowstart = np.concatenate([[0], np.cumsum(C - np.arange(C))]).astype(np.int64)

        def prod(U, V):
            """One [128,512] PSUM bank <- U @ V (both [P,512] fp16, symmetric)."""
            p_t = psum.tile([P, 2 * C], F32, tag="ps_big")
            for mt in range(2):
                oc = slice(mt * C, (mt + 1) * C)
                ms0 = slice(mt * P, mt * P + P)
                ms1 = slice(C + mt * P, C + mt * P + P)
                nc.tensor.matmul(
                    p_t[:, oc], U[:, ms0], V[:, 0:C], start=True, stop=False
                )
                nc.tensor.matmul(
                    p_t[:, oc], U[:, ms1], V[:, C : 2 * C], start=False, stop=True
                )
            return p_t

        def sample_stages(b):
            """Yield closures for one sample's pipeline stages; tiles tagged
            by b%2 so a pair of samples uses disjoint pool slots and their
            PE bursts interleave (keeps the PE dense enough to stay warm)."""
            x = {}
            fx = f"_{b % 2}"

            def load():
                x["xr"] = work.tile([P, 2, M], F32, tag="xr" + fx, name="xr" + fx)
                nc.sync.dma_start(
                    x["xr"][:], x_ap[b].rearrange("(h p) m -> p h m", p=P)
                )

            def stats():
                xr = x["xr"]
                mean2 = work.tile([P, 2], F32, tag="mean2" + fx, name="mean2" + fx)
                nc.vector.tensor_reduce(
                    mean2[:], xr[:], axis=mybir.AxisListType.X,
                    op=mybir.AluOpType.add,
                )
                negmean = work.tile([P, 2], F32, tag="negmean" + fx, name="nm" + fx)
                nc.vector.tensor_scalar_mul(negmean[:], mean2[:], -1.0 / M)
                xc = work.tile([P, 2, M], MM_DT, tag="xc" + fx, name="xc" + fx)
                sq = work.tile([P, 2, M], MM_DT, tag="sq" + fx, name="sq" + fx)
                s2 = work.tile([P, 2], F32, tag="s2" + fx, name="s2" + fx)
                for h in range(2):
                    nc.scalar.activation(
                        xc[:, h], xr[:, h],
                        mybir.ActivationFunctionType.Identity,
                        bias=negmean[:, h : h + 1],
                    )
                    nc.scalar.activation(
                        sq[:, h], xc[:, h],
                        mybir.ActivationFunctionType.Square,
                        accum_out=s2[:, h : h + 1],
                    )
                x["xc"], x["s2"] = xc, s2

            def trace():
                s2 = x["s2"]
                tr_ps = psum_small.tile([P, 1], F32, tag="ps_small", name="tr" + fx)
                nc.tensor.matmul(
                    tr_ps[:], s2[:, 0:1].broadcast_to([P, P]), ones[:],
                    start=True, stop=False,
                )
                nc.tensor.matmul(
                    tr_ps[:], s2[:, 1:2].broadcast_to([P, P]), ones[:],
                    start=False, stop=True,
                )
                abv = work.tile([P, 2], F32, tag="abv" + fx, name="abv" + fx)
                inv = work.tile([P, 1], F32, tag="inv" + fx, name="inv" + fx)
                nc.vector.reciprocal(inv[:], tr_ps[:])
                nc.scalar.activation(
                    abv[:, 0:1], inv[:], mybir.ActivationFunctionType.Sqrt,
                    scale=1.0,
                )
                nc.scalar.activation(
                    abv[:, 1:2], tr_ps[:], mybir.ActivationFunctionType.Sqrt,
                    scale=1.0 / (M * 8192.0 * 8192.0),
                )
                x["abv"] = abv

            def transpose():
                xc = x["xc"]
                tp = psum.tile([P, 2 * C], MM_DT, tag="ps_big", name="tp" + fx)
                for h in range(2):
                    nc.tensor.transpose(
                        tp[:, h * P : h * P + P], xc[:, h, 0:P], ident[:]
                    )
                    nc.tensor.transpose(
                        tp[0 : M - P, C + h * P : C + h * P + P], xc[:, h, P:M],
                        ident[:],
                    )
                x["tp"] = tp

            def scale_xcT():
                tp, abv = x["tp"], x["abv"]
                xcT0 = work.tile([P, C], MM_DT, tag="xcT0" + fx, name="xcT0" + fx)
                xcT1 = work.tile([P, C], MM_DT, tag="xcT1" + fx, name="xcT1" + fx)
                nc.vector.tensor_scalar_mul(xcT0[:], tp[:, 0:C], abv[:, 0:1])
                nc.vector.tensor_scalar_mul(
                    xcT1[0 : M - P], tp[0 : M - P, C : 2 * C],
                    abv[0 : M - P, 0:1],
                )
                x["xcT0"], x["xcT1"] = xcT0, xcT1

            def gram():
                xcT0, xcT1 = x["xcT0"], x["xcT1"]
                a_ps = psum.tile([P, 2 * C], F32, tag="ps_big", name="aps" + fx)
                for mt in range(2):
                    oc = slice(mt * C, (mt + 1) * C)
                    ms = slice(mt * P, (mt + 1) * P)
                    nc.tensor.matmul(
                        a_ps[:, oc], xcT0[:, ms], xcT0[:], start=True, stop=False
                    )
                    nc.tensor.matmul(
                        a_ps[:, oc], xcT1[0 : M - P, ms], xcT1[0 : M - P, :],
                        start=False, stop=True,
                    )
                x["a_ps"] = a_ps

            def mat(tag):
                t = mats.tile([P, 2 * C], MM_DT, tag=tag + fx, name=tag + fx)
                x[tag] = t
                return t

            def drain_A():
                nc.scalar.activation(
                    mat("A")[:], x["a_ps"][:], mybir.ActivationFunctionType.Copy
                )

            def zy1():
                nc.vector.tensor_tensor(
                    mat("ZY1")[:], icons[:, 0, :], x["A"][:],
                    op=mybir.AluOpType.subtract,
                )

            def mk_prod(dst, u, v):
                def f():
                    x[dst] = prod(x[u], x[v])
                return f

            def drain(dst, src, eng):
                def f():
                    t = mat(dst)
                    if eng == "act":
                        nc.scalar.activation(
                            t[:], x[src][:], mybir.ActivationFunctionType.Copy
                        )
                    else:
                        nc.vector.tensor_copy(t[:], x[src][:])
                return f

            def sub(dst, k, src):
                def f():
                    nc.vector.tensor_tensor(
                        mat(dst)[:], icons[:, k, :], x[src][:],
                        op=mybir.AluOpType.subtract,
                    )
                return f

            def fstore():
                f_ps, abv = x["f_ps"], x["abv"]
                for mt in range(2):
                    nc.vector.tensor_scalar_mul(
                        ftiles[mt][:, b, :], f_ps[:, mt * C : (mt + 1) * C],
                        abv[:, 1:2],
                    )

            return [
                load, stats, trace, transpose, scale_xcT, gram,
                drain_A, zy1,
                mk_prod("y1_ps", "A", "ZY1"), drain("Y1", "y1_ps", "act"),
                mk_prod("w1_ps", "ZY1", "Y1"), sub("ZY2", 1, "w1_ps"),
                mk_prod("y2_ps", "Y1", "ZY2"), drain("Y2", "y2_ps", "dve"),
                mk_prod("z2_ps", "ZY2", "ZY1"), drain("Z2", "z2_ps", "act"),
                mk_prod("w2_ps", "Z2", "Y2"), sub("ZY3", 2, "w2_ps"),
                mk_prod("f_ps", "Y2", "ZY3"), fstore,
            ]

        for b0 in range(0, n_samples, 2):
            stages = [sample_stages(b0)]
            if b0 + 1 < n_samples:
                stages.append(sample_stages(b0 + 1))
            for step in range(len(stages[0])):
                for sg in stages:
                    sg[step]()

        # ---- flush: one DMA per matrix row, all samples at once ----
        for r in range(C):
            L = C - r
            s0 = int(rowstart[r])
            src = ftiles[r // P][r % P : r % P + 1, :, r:C]
            eng = nc.sync if r % 2 == 0 else nc.scalar
            eng.dma_start(y_ap[:, s0 : s0 + L], src)


def _make_const_inputs():
    # icons[:, k, :]: [3I, 12I, 768I] in concatenated row-tile layout:
    # cols 0:256 = matrix rows 0:128 (diag at col p),
    # cols 256:512 = matrix rows 128:256 (diag at col 256+128+p).
    e = np.zeros((P, 2 * C), np.float32)
    e[np.arange(P), np.arange(P)] = 1.0
    e[np.arange(P), C + P + np.arange(P)] = 1.0
    icons = np.stack([3.0 * e, 12.0 * e, 768.0 * e], axis=1).astype(np.float16)
    return {
        "ident": np.eye(P, dtype=np.float16),
        "icons": np.ascontiguousarray(icons),
        "ones": np.ones((P, 1), np.float32),
        "onesrow": np.ones((1, P), np.float32),
    }


def make_nc(n_samples=S, num_devices=NCORES):
    nc = bacc.Bacc(
        "TRN2",
        target_bir_lowering=False,
        debug=False,
        enable_asserts=False,
        num_devices=num_devices,
    )
    x_ap = nc.dram_tensor("x", (n_samples, C, M), F32, kind="ExternalInput").ap()
    y_ap = nc.dram_tensor("y", (n_samples, NTRIU), F32, kind="ExternalOutput").ap()
    ident_ap = nc.dram_tensor("ident", (P, P), MM_DT, kind="ExternalInput").ap()
    icons_ap = nc.dram_tensor("icons", (P, 3, 2 * C), MM_DT, kind="ExternalInput").ap()
    ones_ap = nc.dram_tensor("ones", (P, 1), F32, kind="ExternalInput").ap()
    onesrow_ap = nc.dram_tensor("onesrow", (1, P), F32, kind="ExternalInput").ap()
    with tile.TileContext(nc) as tc:
        build(tc, y_ap, x_ap, ident_ap, icons_ap, ones_ap, onesrow_ap, n_samples)
    nc.compile()
    return nc


def kernel(x, _trace=False, **_trace_kwargs):
    global LAST_EXEC_NS, LAST_RESULTS
    x = np.ascontiguousarray(np.asarray(x), dtype=np.float32)
    assert x.shape == (B, C, 14, 14)
    xr = x.reshape(B, C, M)

    nc = make_nc()
    consts = _make_const_inputs()
    in_maps = [
        {"x": np.ascontiguousarray(xr[i * S : (i + 1) * S]), **consts}
        for i in range(NCORES)
    ]
    res = bass_utils.run_bass_kernel_spmd(
        nc, in_maps, core_ids=list(range(NCORES)), trace=_trace, **_trace_kwargs
    )
    LAST_EXEC_NS = res.exec_time_ns
    LAST_RESULTS = res
    return np.concatenate([r["y"] for r in res.results], axis=0)


# revision 14
# speedup vs baseline: 1.7081x; 1.0377x over previous
"""MPNCOV (iSQRT-COV pooling) Trainium2 kernel.

Math per sample (C=256 channels, M=196 spatial):
  xc   = x - mean_m(x)                      # center along spatial dim
  A    = xc @ xc^T / sum(xc^2)              # = cov / trace(cov)
  Newton-Schulz (ITER_N=3) on A, final y = sqrt(normA) * YZY, triu-packed.

Scale folding: every intermediate X is stored as X_s with X = sigma_X * X_s,
sigma tracked symbolically so each PSUM->SBUF transform is a single
tensor_tensor subtract against a constant diagonal tile:
  ZY1_s = 3I   - A_s          (sigma 1/2)
  Y1_s  = A_s @ ZY1_s         (sigma 1/2)
  W1_s  = ZY1_s @ Y1_s        (sigma 1/4)
  ZY2_s = 12I  - W1_s         (sigma 1/8)
  Y2_s  = Y1_s @ ZY2_s        (sigma 1/16)
  Z2_s  = ZY2_s @ ZY1_s       (sigma 1/16)
  W2_s  = Z2_s @ Y2_s         (sigma 1/256)
  ZY3_s = 768I - W2_s
  F_s   = Y2_s @ ZY3_s,   y = (sqrt(tr/M)/8192) * F_s
All intermediates are polynomials in symmetric A => symmetric, so row-tiles
serve directly as matmul lhsT (no transposes in the NS chain). The only PE
transposes build xc^T for the Gram matmul; 1/sqrt(sum xc^2) is folded into
the transpose's PSUM->SBUF copy so the Gram directly yields A_s.

Matrices are stored as single [128, 512] tiles: cols 0:256 = matrix rows
0:128, cols 256:512 = matrix rows 128:256. Each product lands in ONE fp32
PSUM bank [128, 512] (two N=256 matmul groups), so every PSUM->SBUF
transform is one 512-wide DVE/ACT op. Matmul inputs are fp16 (1 cyc/row on
the PE + fast weight load); PSUM accumulation stays fp32.

Sharding: pure data parallel, batch 256 -> 32 samples on each of 8 cores.
Triu packing: all 32 per-sample results stay SBUF-resident; at the end one
DMA per matrix row r moves that row's triu tail for all 32 samples
(constant strides in both src and dst), alternating sync/scalar HWDGE.
"""

import numpy as np

from concourse import bacc, bass, mybir, tile
from concourse import bass_utils

F32 = mybir.dt.float32
P = 128
C = 256
M = 196
B = 256
NCORES = 8
S = B // NCORES            # samples per core
NTRIU = C * (C + 1) // 2   # 32896

# matmul input dtype for the big products
MM_DT = mybir.dt.float16

LAST_EXEC_NS = None
LAST_RESULTS = None


def build(tc, y_ap, x_ap, ident_ap, icons_ap, ones_ap, onesrow_ap, n_samples=S):
    nc = tc.nc
    import contextlib

    with contextlib.ExitStack() as ctx:
        consts = ctx.enter_context(tc.tile_pool(name="consts", bufs=1))
        fpool = ctx.enter_context(tc.tile_pool(name="fpool", bufs=1))
        work = ctx.enter_context(tc.tile_pool(name="work", bufs=3))
        mats = ctx.enter_context(tc.tile_pool(name="mats", bufs=2))
        psum = ctx.enter_context(tc.tile_pool(name="psum", bufs=7, space="PSUM"))
        psum_small = ctx.enter_context(
            tc.tile_pool(name="psum_small", bufs=1, space="PSUM")
        )

        ident = consts.tile([P, P], MM_DT, tag="ident")
        nc.sync.dma_start(ident[:], ident_ap[:])
        icons = consts.tile([P, 3, 2 * C], MM_DT, tag="icons")
        nc.sync.dma_start(icons[:], icons_ap[:])
        ones = consts.tile([P, 1], F32, tag="ones")
        nc.sync.dma_start(ones[:], ones_ap[:])
        onesrow = consts.tile([1, P], F32, tag="onesrow")
        nc.sync.dma_start(onesrow[:], onesrow_ap[:])

        ftiles = [
            fpool.tile([P, n_samples, C], F32, tag=f"F_m{mt}", name=f"F_m{mt}")
            for mt in range(2)
        ]

        rowstart = np.concatenate([[0], np.cumsum(C - np.arange(C))]).astype(np.int64)

        def prod(U, V):
            """One [128,512] PSUM bank <- U @ V (both [P,512] fp16, symmetric)."""
            p_t = psum.tile([P, 2 * C], F32, tag="ps_big")
            for mt in range(2):
                oc = slice(mt * C, (mt + 1) * C)
                ms0 = slice(mt * P, mt * P + P)
                ms1 = slice(C + mt * P, C + mt * P + P)
                nc.tensor.matmul(
                    p_t[:, oc], U[:, ms0], V[:, 0:C], start=True, stop=False
                )
                nc.tensor.matmul(
                    p_t[:, oc], U[:, ms1], V[:, C : 2 * C], start=False, stop=True
                )
            return p_t

        def sample_stages(b):
            """Yield closures for one sample's pipeline stages; tiles tagged
            by b%2 so a pair of samples uses disjoint pool slots and their
            PE bursts interleave (keeps the PE dense enough to stay warm)."""
            x = {}
            fx = f"_{b % 2}"

            def load():
                x["xr"] = work.tile([P, 2, M], F32, tag="xr" + fx, name="xr" + fx)
                nc.sync.dma_start(
                    x["xr"][:], x_ap[b].rearrange("(h p) m -> p h m", p=P)
                )

            def stats():
                xr = x["xr"]
                mean2 = work.tile([P, 2], F32, tag="mean2" + fx, name="mean2" + fx)
                nc.vector.tensor_reduce(
                    mean2[:], xr[:], axis=mybir.AxisListType.X,
                    op=mybir.AluOpType.add,
                )
                negmean = work.tile([P, 2], F32, tag="negmean" + fx, name="nm" + fx)
                nc.vector.tensor_scalar_mul(negmean[:], mean2[:], -1.0 / M)
                xc = work.tile([P, 2, M], MM_DT, tag="xc" + fx, name="xc" + fx)
                sq = work.tile([P, 2, M], MM_DT, tag="sq" + fx, name="sq" + fx)
                s2 = work.tile([P, 2], F32, tag="s2" + fx, name="s2" + fx)
                for h in range(2):
                    nc.scalar.activation(
                        xc[:, h], xr[:, h],
                        mybir.ActivationFunctionType.Identity,
                        bias=negmean[:, h : h + 1],
                    )
                    nc.scalar.activation(
                        sq[:, h], xc[:, h],
                        mybir.ActivationFunctionType.Square,
                        accum_out=s2[:, h : h + 1],
                    )
                x["xc"], x["s2"] = xc, s2

            def trace():
                s2 = x["s2"]
                s2s = work.tile([P, 1], F32, tag="s2s" + fx, name="s2s" + fx)
                nc.vector.tensor_tensor(
                    s2s[:], s2[:, 0:1], s2[:, 1:2], op=mybir.AluOpType.add
                )
                tr_ps = psum_small.tile([P, 1], F32, tag="ps_small", name="tr" + fx)
                nc.tensor.matmul(
                    tr_ps[:], s2s[:].broadcast_to([P, P]), ones[:],
                    start=True, stop=True,
                )
                abv = work.tile([P, 2], F32, tag="abv" + fx, name="abv" + fx)
                inv = work.tile([P, 1], F32, tag="inv" + fx, name="inv" + fx)
                nc.vector.reciprocal(inv[:], tr_ps[:])
                nc.scalar.activation(
                    abv[:, 0:1], inv[:], mybir.ActivationFunctionType.Sqrt,
                    scale=1.0,
                )
                nc.scalar.activation(
                    abv[:, 1:2], tr_ps[:], mybir.ActivationFunctionType.Sqrt,
                    scale=1.0 / (M * 8192.0 * 8192.0),
                )
                x["abv"] = abv

            def transpose():
                xc = x["xc"]
                tp = psum.tile([P, 2 * C], MM_DT, tag="ps_big", name="tp" + fx)
                for h in range(2):
                    nc.tensor.transpose(
                        tp[:, h * P : h * P + P], xc[:, h, 0:P], ident[:]
                    )
                    nc.tensor.transpose(
                        tp[0 : M - P, C + h * P : C + h * P + P], xc[:, h, P:M],
                        ident[:],
                    )
                x["tp"] = tp

            def scale_xcT():
                tp, abv = x["tp"], x["abv"]
                xcT0 = work.tile([P, C], MM_DT, tag="xcT0" + fx, name="xcT0" + fx)
                xcT1 = work.tile([P, C], MM_DT, tag="xcT1" + fx, name="xcT1" + fx)
                nc.vector.tensor_scalar_mul(xcT0[:], tp[:, 0:C], abv[:, 0:1])
                nc.vector.tensor_scalar_mul(
                    xcT1[0 : M - P], tp[0 : M - P, C : 2 * C],
                    abv[0 : M - P, 0:1],
                )
                x["xcT0"], x["xcT1"] = xcT0, xcT1

            def gram():
                xcT0, xcT1 = x["xcT0"], x["xcT1"]
                a_ps = psum.tile([P, 2 * C], F32, tag="ps_big", name="aps" + fx)
                for mt in range(2):
                    oc = slice(mt * C, (mt + 1) * C)
                    ms = slice(mt * P, (mt + 1) * P)
                    nc.tensor.matmul(
                        a_ps[:, oc], xcT0[:, ms], xcT0[:], start=True, stop=False
                    )
                    nc.tensor.matmul(
                        a_ps[:, oc], xcT1[0 : M - P, ms], xcT1[0 : M - P, :],
                        start=False, stop=True,
                    )
                x["a_ps"] = a_ps

            def mat(tag):
                t = mats.tile([P, 2 * C], MM_DT, tag=tag + fx, name=tag + fx)
                x[tag] = t
                return t

            def drain_A():
                nc.scalar.activation(
                    mat("A")[:], x["a_ps"][:], mybir.ActivationFunctionType.Copy
                )

            def zy1():
                nc.vector.tensor_tensor(
                    mat("ZY1")[:], icons[:, 0, :], x["A"][:],
                    op=mybir.AluOpType.subtract,
                )

            def mk_prod(dst, u, v):
                def f():
                    x[dst] = prod(x[u], x[v])
                return f

            def drain(dst, src, eng):
                def f():
                    t = mat(dst)
                    if eng == "act":
                        nc.scalar.activation(
                            t[:], x[src][:], mybir.ActivationFunctionType.Copy
                        )
                    else:
                        nc.vector.tensor_copy(t[:], x[src][:])
                return f

            def sub(dst, k, src):
                def f():
                    nc.vector.tensor_tensor(
                        mat(dst)[:], icons[:, k, :], x[src][:],
                        op=mybir.AluOpType.subtract,
                    )
                return f

            def fstore():
                f_ps, abv = x["f_ps"], x["abv"]
                for mt in range(2):
                    nc.vector.tensor_scalar_mul(
                        ftiles[mt][:, b, :], f_ps[:, mt * C : (mt + 1) * C],
                        abv[:, 1:2],
                    )

            return [
                load, stats, trace, transpose, scale_xcT, gram,
                drain_A, zy1,
                mk_prod("y1_ps", "A", "ZY1"), drain("Y1", "y1_ps", "act"),
                mk_prod("w1_ps", "ZY1", "Y1"), sub("ZY2", 1, "w1_ps"),
                mk_prod("y2_ps", "Y1", "ZY2"), drain("Y2", "y2_ps", "dve"),
                mk_prod("z2_ps", "ZY2", "ZY1"), drain("Z2", "z2_ps", "act"),
                mk_prod("w2_ps", "Z2", "Y2"), sub("ZY3", 2, "w2_ps"),
                mk_prod("f_ps", "Y2", "ZY3"), fstore,
            ]

        for b0 in range(0, n_samples, 2):
            sa = sample_stages(b0)
            sb = sample_stages(b0 + 1) if b0 + 1 < n_samples else []
            n = len(sa)
            for step in range(n + 1):
                if step < n:
                    sa[step]()
                if sb and step >= 1:
                    sb[step - 1]()

        # ---- flush: one DMA per matrix row, all samples at once ----
        for r in range(C):
            L = C - r
            s0 = int(rowstart[r])
            src = ftiles[r // P][r % P : r % P + 1, :, r:C]
            if r % 6 == 5:
                eng = nc.gpsimd
            elif r % 2 == 0:
                eng = nc.sync
            else:
                eng = nc.scalar
            eng.dma_start(y_ap[:, s0 : s0 + L], src)


def _make_const_inputs():
    # icons[:, k, :]: [3I, 12I, 768I] in concatenated row-tile layout:
    # cols 0:256 = matrix rows 0:128 (diag at col p),
    # cols 256:512 = matrix rows 128:256 (diag at col 256+128+p).
    e = np.zeros((P, 2 * C), np.float32)
    e[np.arange(P), np.arange(P)] = 1.0
    e[np.arange(P), C + P + np.arange(P)] = 1.0
    icons = np.stack([3.0 * e, 12.0 * e, 768.0 * e], axis=1).astype(np.float16)
    return {
        "ident": np.eye(P, dtype=np.float16),
        "icons": np.ascontiguousarray(icons),
        "ones": np.ones((P, 1), np.float32),
        "onesrow": np.ones((1, P), np.float32),
    }


def make_nc(n_samples=S, num_devices=NCORES):
    nc = bacc.Bacc(
        "TRN2",
        target_bir_lowering=False,
        debug=False,
        enable_asserts=False,
        num_devices=num_devices,
    )
    x_ap = nc.dram_tensor("x", (n_samples, C, M), F32, kind="ExternalInput").ap()
    y_ap = nc.dram_tensor("y", (n_samples, NTRIU), F32, kind="ExternalOutput").ap()
    ident_ap = nc.dram_tensor("ident", (P, P), MM_DT, kind="ExternalInput").ap()
    icons_ap = nc.dram_tensor("icons", (P, 3, 2 * C), MM_DT, kind="ExternalInput").ap()
    ones_ap = nc.dram_tensor("ones", (P, 1), F32, kind="ExternalInput").ap()
    onesrow_ap = nc.dram_tensor("onesrow", (1, P), F32, kind="ExternalInput").ap()
    with tile.TileContext(nc) as tc:
        build(tc, y_ap, x_ap, ident_ap, icons_ap, ones_ap, onesrow_ap, n_samples)
    nc.compile()
    return nc


def kernel(x, _trace=False, **_trace_kwargs):
    global LAST_EXEC_NS, LAST_RESULTS
    x = np.ascontiguousarray(np.asarray(x), dtype=np.float32)
    assert x.shape == (B, C, 14, 14)
    xr = x.reshape(B, C, M)

    nc = make_nc()
    consts = _make_const_inputs()
    in_maps = [
        {"x": np.ascontiguousarray(xr[i * S : (i + 1) * S]), **consts}
        for i in range(NCORES)
    ]
    res = bass_utils.run_bass_kernel_spmd(
        nc, in_maps, core_ids=list(range(NCORES)), trace=_trace, **_trace_kwargs
    )
    LAST_EXEC_NS = res.exec_time_ns
    LAST_RESULTS = res
    return np.concatenate([r["y"] for r in res.results], axis=0)


# revision 20
# speedup vs baseline: 2.2197x; 1.2995x over previous
"""MPNCOV (iSQRT-COV pooling) Trainium2 kernel.

Math per sample (C=256 channels, M=196 spatial):
  xc   = x - mean_m(x)                      # center along spatial dim
  A    = xc @ xc^T / sum(xc^2)              # = cov / trace(cov)
  Newton-Schulz (ITER_N=3) on A, final y = sqrt(normA) * YZY, triu-packed.

Scale folding: every intermediate X is stored as X_s with X = sigma_X * X_s,
sigma tracked symbolically so each PSUM->SBUF transform is a single
tensor_tensor subtract against a constant diagonal tile:
  ZY1_s = 3I   - A_s          (sigma 1/2)
  Y1_s  = A_s @ ZY1_s         (sigma 1/2)
  W1_s  = ZY1_s @ Y1_s        (sigma 1/4)
  ZY2_s = 12I  - W1_s         (sigma 1/8)
  Y2_s  = Y1_s @ ZY2_s        (sigma 1/16)
  Z2_s  = ZY2_s @ ZY1_s       (sigma 1/16)
  W2_s  = Z2_s @ Y2_s         (sigma 1/256)
  ZY3_s = 768I - W2_s
  F_s   = Y2_s @ ZY3_s,   y = (sqrt(tr/M)/8192) * F_s
All intermediates are polynomials in symmetric A => symmetric, so row-tiles
serve directly as matmul lhsT (no transposes in the NS chain). The only PE
transposes build xc^T for the Gram matmul; 1/sqrt(sum xc^2) is folded into
the transpose's PSUM->SBUF copy so the Gram directly yields A_s.

Matrices are stored as single [128, 512] tiles: cols 0:256 = matrix rows
0:128, cols 256:512 = matrix rows 128:256. Each product lands in ONE fp32
PSUM bank [128, 512] (two N=256 matmul groups), so every PSUM->SBUF
transform is one 512-wide DVE/ACT op. Matmul inputs are fp16 (1 cyc/row on
the PE + fast weight load); PSUM accumulation stays fp32.

Sharding: pure data parallel, batch 256 -> 32 samples on each of 8 cores.
Triu packing: all 32 per-sample results stay SBUF-resident; at the end one
DMA per matrix row r moves that row's triu tail for all 32 samples
(constant strides in both src and dst), alternating sync/scalar HWDGE.
"""

import numpy as np

from concourse import bacc, bass, bass_isa, mybir, tile
from concourse import bass_utils

F32 = mybir.dt.float32
P = 128
C = 256
M = 196
B = 256
NCORES = 8
S = B // NCORES            # samples per core
NTRIU = C * (C + 1) // 2   # 32896

# matmul input dtype for the big products
MM_DT = mybir.dt.float16

LAST_EXEC_NS = None
LAST_RESULTS = None


def build(tc, y_ap, x_ap, ident_ap, icons_ap, ones_ap, onesrow_ap, n_samples=S):
    nc = tc.nc
    import contextlib

    with contextlib.ExitStack() as ctx:
        consts = ctx.enter_context(tc.tile_pool(name="consts", bufs=1))
        fpool = ctx.enter_context(tc.tile_pool(name="fpool", bufs=1))
        work = ctx.enter_context(tc.tile_pool(name="work", bufs=3))
        mats = ctx.enter_context(tc.tile_pool(name="mats", bufs=2))
        psum = ctx.enter_context(tc.tile_pool(name="psum", bufs=8, space="PSUM"))

        ident = consts.tile([P, P], MM_DT, tag="ident")
        nc.sync.dma_start(ident[:], ident_ap[:])
        icons = consts.tile([P, 3, 2 * C], MM_DT, tag="icons")
        nc.sync.dma_start(icons[:], icons_ap[:])
        ones = consts.tile([P, 1], F32, tag="ones")
        nc.sync.dma_start(ones[:], ones_ap[:])
        onesrow = consts.tile([1, P], F32, tag="onesrow")
        nc.sync.dma_start(onesrow[:], onesrow_ap[:])

        ftiles = [
            fpool.tile([P, n_samples, C], F32, tag=f"F_m{mt}", name=f"F_m{mt}")
            for mt in range(2)
        ]

        rowstart = np.concatenate([[0], np.cumsum(C - np.arange(C))]).astype(np.int64)

        def prod(U, V):
            """One [128,512] PSUM bank <- U @ V (both [P,512] fp16, symmetric)."""
            p_t = psum.tile([P, 2 * C], F32, tag="ps_big")
            for mt in range(2):
                oc = slice(mt * C, (mt + 1) * C)
                ms0 = slice(mt * P, mt * P + P)
                ms1 = slice(C + mt * P, C + mt * P + P)
                nc.tensor.matmul(
                    p_t[:, oc], U[:, ms0], V[:, 0:C], start=True, stop=False
                )
                nc.tensor.matmul(
                    p_t[:, oc], U[:, ms1], V[:, C : 2 * C], start=False, stop=True
                )
            return p_t

        def sample_stages(b):
            """Yield closures for one sample's pipeline stages; tiles tagged
            by b%2 so a pair of samples uses disjoint pool slots and their
            PE bursts interleave (keeps the PE dense enough to stay warm)."""
            x = {}
            fx = f"_{b % 3}"

            def load():
                x["xr"] = work.tile([P, 2, M], F32, tag="xr" + fx, name="xr" + fx)
                nc.sync.dma_start(
                    x["xr"][:], x_ap[b].rearrange("(h p) m -> p h m", p=P)
                )

            def stats():
                xr = x["xr"]
                mean2 = work.tile([P, 2], F32, tag="mean2" + fx, name="mean2" + fx)
                nc.vector.tensor_reduce(
                    mean2[:], xr[:], axis=mybir.AxisListType.X,
                    op=mybir.AluOpType.add,
                )
                negmean = work.tile([P, 2], F32, tag="negmean" + fx, name="nm" + fx)
                nc.vector.tensor_scalar_mul(negmean[:], mean2[:], -1.0 / M)
                xc = work.tile([P, 2, M], MM_DT, tag="xc" + fx, name="xc" + fx)
                sq = work.tile([P, 2, M], MM_DT, tag="sq" + fx, name="sq" + fx)
                s2 = work.tile([P, 2], F32, tag="s2" + fx, name="s2" + fx)
                for h in range(2):
                    nc.scalar.activation(
                        xc[:, h], xr[:, h],
                        mybir.ActivationFunctionType.Identity,
                        bias=negmean[:, h : h + 1],
                    )
                    nc.scalar.activation(
                        sq[:, h], xc[:, h],
                        mybir.ActivationFunctionType.Square,
                        accum_out=s2[:, h : h + 1],
                    )
                x["xc"], x["s2"] = xc, s2

            def trace():
                s2 = x["s2"]
                s2r = work.tile([P, 2], F32, tag="s2r" + fx, name="s2r" + fx)
                nc.gpsimd.partition_all_reduce(
                    s2r[:], s2[:], channels=P, reduce_op=bass_isa.ReduceOp.add
                )
                trv = work.tile([P, 1], F32, tag="trv" + fx, name="trv" + fx)
                nc.vector.tensor_tensor(
                    trv[:], s2r[:, 0:1], s2r[:, 1:2], op=mybir.AluOpType.add
                )
                abv = work.tile([P, 2], F32, tag="abv" + fx, name="abv" + fx)
                inv = work.tile([P, 1], F32, tag="inv" + fx, name="inv" + fx)
                nc.vector.reciprocal(inv[:], trv[:])
                nc.scalar.activation(
                    abv[:, 0:1], inv[:], mybir.ActivationFunctionType.Sqrt,
                    scale=1.0,
                )
                nc.scalar.activation(
                    abv[:, 1:2], trv[:], mybir.ActivationFunctionType.Sqrt,
                    scale=1.0 / (M * 8192.0 * 8192.0),
                )
                x["abv"] = abv

            def transpose():
                xc = x["xc"]
                tp = psum.tile([P, 2 * C], MM_DT, tag="ps_big", name="tp" + fx)
                for h in range(2):
                    nc.tensor.transpose(
                        tp[:, h * P : h * P + P], xc[:, h, 0:P], ident[:]
                    )
                    nc.tensor.transpose(
                        tp[0 : M - P, C + h * P : C + h * P + P], xc[:, h, P:M],
                        ident[:],
                    )
                x["tp"] = tp

            def scale_xcT():
                tp, abv = x["tp"], x["abv"]
                xcT0 = work.tile([P, C], MM_DT, tag="xcT0" + fx, name="xcT0" + fx)
                xcT1 = work.tile([P, C], MM_DT, tag="xcT1" + fx, name="xcT1" + fx)
                nc.vector.tensor_scalar_mul(xcT0[:], tp[:, 0:C], abv[:, 0:1])
                nc.vector.tensor_scalar_mul(
                    xcT1[0 : M - P], tp[0 : M - P, C : 2 * C],
                    abv[0 : M - P, 0:1],
                )
                x["xcT0"], x["xcT1"] = xcT0, xcT1

            def gram():
                xcT0, xcT1 = x["xcT0"], x["xcT1"]
                a_ps = psum.tile([P, 2 * C], F32, tag="ps_big", name="aps" + fx)
                for mt in range(2):
                    oc = slice(mt * C, (mt + 1) * C)
                    ms = slice(mt * P, (mt + 1) * P)
                    nc.tensor.matmul(
                        a_ps[:, oc], xcT0[:, ms], xcT0[:], start=True, stop=False
                    )
                    nc.tensor.matmul(
                        a_ps[:, oc], xcT1[0 : M - P, ms], xcT1[0 : M - P, :],
                        start=False, stop=True,
                    )
                x["a_ps"] = a_ps

            def mat(tag):
                t = mats.tile([P, 2 * C], MM_DT, tag=tag + fx, name=tag + fx)
                x[tag] = t
                return t

            def drain_A():
                nc.scalar.activation(
                    mat("A")[:], x["a_ps"][:], mybir.ActivationFunctionType.Copy
                )

            def zy1():
                nc.vector.tensor_tensor(
                    mat("ZY1")[:], icons[:, 0, :], x["A"][:],
                    op=mybir.AluOpType.subtract,
                )

            def mk_prod(dst, u, v):
                def f():
                    x[dst] = prod(x[u], x[v])
                return f

            def drain(dst, src, eng):
                def f():
                    t = mat(dst)
                    if eng == "act":
                        nc.scalar.activation(
                            t[:], x[src][:], mybir.ActivationFunctionType.Copy
                        )
                    else:
                        nc.vector.tensor_copy(t[:], x[src][:])
                return f

            def sub(dst, k, src):
                def f():
                    nc.vector.tensor_tensor(
                        mat(dst)[:], icons[:, k, :], x[src][:],
                        op=mybir.AluOpType.subtract,
                    )
                return f

            def fstore():
                f_ps, abv = x["f_ps"], x["abv"]
                nc.vector.tensor_scalar_mul(
                    ftiles[0][:, b, :], f_ps[:, 0:C], abv[:, 1:2]
                )
                nc.scalar.activation(
                    ftiles[1][:, b, :], f_ps[:, C : 2 * C],
                    mybir.ActivationFunctionType.Copy, scale=abv[:, 1:2],
                )

            return [
                load, stats, trace, transpose, scale_xcT, gram,
                drain_A, zy1,
                mk_prod("y1_ps", "A", "ZY1"), drain("Y1", "y1_ps", "act"),
                mk_prod("w1_ps", "ZY1", "Y1"), sub("ZY2", 1, "w1_ps"),
                mk_prod("y2_ps", "Y1", "ZY2"), drain("Y2", "y2_ps", "dve"),
                mk_prod("z2_ps", "ZY2", "ZY1"), drain("Z2", "z2_ps", "act"),
                mk_prod("w2_ps", "Z2", "Y2"), sub("ZY3", 2, "w2_ps"),
                mk_prod("f_ps", "Y2", "ZY3"), fstore,
            ]

        for b0 in range(0, n_samples, 3):
            grp = [sample_stages(b) for b in range(b0, min(b0 + 3, n_samples))]
            n = len(grp[0])
            for step in range(n + 2):
                for i, sg in enumerate(grp):
                    if 0 <= step - i < n:
                        sg[step - i]()

        # ---- flush: one DMA per matrix row, all samples at once ----
        for r in range(C):
            L = C - r
            s0 = int(rowstart[r])
            src = ftiles[r // P][r % P : r % P + 1, :, r:C]
            # measured issue rates: gpsimd 0.59us, sync 0.77us, scalar 0.83us
            m = r % 10
            if m in (0, 2, 4, 6):
                eng = nc.gpsimd
            elif m in (1, 5, 8):
                eng = nc.sync
            else:
                eng = nc.scalar
            eng.dma_start(y_ap[:, s0 : s0 + L], src)


def _make_const_inputs():
    # icons[:, k, :]: [3I, 12I, 768I] in concatenated row-tile layout:
    # cols 0:256 = matrix rows 0:128 (diag at col p),
    # cols 256:512 = matrix rows 128:256 (diag at col 256+128+p).
    e = np.zeros((P, 2 * C), np.float32)
    e[np.arange(P), np.arange(P)] = 1.0
    e[np.arange(P), C + P + np.arange(P)] = 1.0
    icons = np.stack([3.0 * e, 12.0 * e, 768.0 * e], axis=1).astype(np.float16)
    return {
        "ident": np.eye(P, dtype=np.float16),
        "icons": np.ascontiguousarray(icons),
        "ones": np.ones((P, 1), np.float32),
        "onesrow": np.ones((1, P), np.float32),
    }


def make_nc(n_samples=S, num_devices=NCORES):
    nc = bacc.Bacc(
        "TRN2",
        target_bir_lowering=False,
        debug=False,
        enable_asserts=False,
        num_devices=num_devices,
    )
    x_ap = nc.dram_tensor("x", (n_samples, C, M), F32, kind="ExternalInput").ap()
    y_ap = nc.dram_tensor("y", (n_samples, NTRIU), F32, kind="ExternalOutput").ap()
    ident_ap = nc.dram_tensor("ident", (P, P), MM_DT, kind="ExternalInput").ap()
    icons_ap = nc.dram_tensor("icons", (P, 3, 2 * C), MM_DT, kind="ExternalInput").ap()
    ones_ap = nc.dram_tensor("ones", (P, 1), F32, kind="ExternalInput").ap()
    onesrow_ap = nc.dram_tensor("onesrow", (1, P), F32, kind="ExternalInput").ap()
    with tile.TileContext(nc) as tc:
        build(tc, y_ap, x_ap, ident_ap, icons_ap, ones_ap, onesrow_ap, n_samples)
    nc.compile()
    return nc


def kernel(x, _trace=False, **_trace_kwargs):
    global LAST_EXEC_NS, LAST_RESULTS
    x = np.ascontiguousarray(np.asarray(x), dtype=np.float32)
    assert x.shape == (B, C, 14, 14)
    xr = x.reshape(B, C, M)

    nc = make_nc()
    consts = _make_const_inputs()
    in_maps = [
        {"x": np.ascontiguousarray(xr[i * S : (i + 1) * S]), **consts}
        for i in range(NCORES)
    ]
    res = bass_utils.run_bass_kernel_spmd(
        nc, in_maps, core_ids=list(range(NCORES)), trace=_trace, **_trace_kwargs
    )
    LAST_EXEC_NS = res.exec_time_ns
    LAST_RESULTS = res
    return np.concatenate([r["y"] for r in res.results], axis=0)
